# revision 6
# baseline (speedup 1.0000x reference)
"""Trainium2 Bass kernel for nn_Coords2Stress (ANM Hessian + top eigenmode).

Contract: kernel(coords[4,3072] f32, num_atoms[4] int) -> (hess[4,3072,3072] f32,
disp[4,3072] f32), matching reference.py.

Sharding: 8 NeuronCores = 4 samples x 2 halves of 512 atom-rows each.
Each core computes 1536 rows of one sample's 3072x3072 Hessian.

Device math per core (SPMD, per-core data only):
  - j-axis inputs are host-permuted so the core's own 512 atoms come first;
    this makes the diagonal-block window a compile-time constant column range.
  - padded atoms are host-masked to coordinate MASK=1e4 so g underflows to 0.
  - g = exp(-(d2+EPS)/49 - ln(d2+EPS)) computed via ACT Square/Ln/Exp
    (all three functions live in one activation table set).
  - H_ab[i, j] = (x_b[i]-x_b[j]) * ((x_a[j]-x_a[i])*g) via fused
    scalar_tensor_tensor ops with accum_out giving row sums for the
    diagonal blocks, which are then blended in with a -I mask.
Top eigenmode runs on host CPU via jax (LAPACK ssyevd; numpy's LAPACK build
produces opposite eigenvector signs vs jaxlib's, so jax-cpu is required).
"""

import numpy as np

P = 128          # partitions
N = 1024         # atoms per sample
B = 4            # batch
ROWS = 512       # atom-rows per core
NB = ROWS // P   # i-blocks per core
EPS = 1e-5
CUT = 49.0
MASK = 1e4
N_CORES = 8

# const block column offsets
_OFF_CB = 0
_OFF_NCB = 3 * N
_OFF_INEG = 6 * N
_OFF_XYZI = 6 * N + P
_OFF_NXYZI = _OFF_XYZI + 3 * NB
_OFF_M7 = _OFF_NXYZI + 3 * NB
_OFF_EPS = _OFF_M7 + 3 * NB
_OFF_NEGE49 = _OFF_EPS + 1
CF = _OFF_NEGE49 + 1

_CACHE = {}

# Engine placement for the three d2/w adds: "vector" or "gpsimd"
ADDS_ENGINE = "vector"


def _build_nc():
    import concourse.bacc as bacc
    import concourse.mybir as mybir
    import concourse.tile as tile

    f32 = mybir.dt.float32
    AT = mybir.ActivationFunctionType
    OP = mybir.AluOpType

    nc = bacc.Bacc()
    consts = nc.dram_tensor("consts", [P, CF], f32, kind="ExternalInput")
    out = nc.dram_tensor("out", [3 * ROWS, 3 * N], f32, kind="ExternalOutput")

    with tile.TileContext(nc) as tc:
        with tc.tile_pool(name="const", bufs=1) as cpool, \
             tc.tile_pool(name="work", bufs=2) as wpool, \
             tc.tile_pool(name="big", bufs=2) as bpool:
            ct = cpool.tile([P, CF], f32, tag="consts", name="ct")
            nc.sync.dma_start(out=ct[:, :], in_=consts[:, :])

            def cb(a):
                return ct[:, _OFF_CB + N * a:_OFF_CB + N * a + N]

            def ncb(b):
                return ct[:, _OFF_NCB + N * b:_OFF_NCB + N * b + N]

            inegt = ct[:, _OFF_INEG:_OFF_INEG + P]

            def col(off, k, a):
                c = off + 3 * k + a
                return ct[:, c:c + 1]

            eps_b = ct[:, _OFF_EPS:_OFF_EPS + 1]
            nege49_b = ct[:, _OFF_NEGE49:_OFF_NEGE49 + 1]

            add_eng = nc.vector if ADDS_ENGINE == "vector" else nc.gpsimd

            for k in range(NB):
                sq = [wpool.tile([P, N], f32, tag=f"sq{a}", name=f"sq{a}_{k}")
                      for a in range(3)]
                for a in range(3):
                    nc.scalar.activation(
                        sq[a][:, :], cb(a), AT.Square,
                        bias=col(_OFF_M7, k, a), scale=1.0 / 7.0)
                # d2s = (sqx+sqy+sqz), scaled by 1/49 via the Square inputs
                d2s = wpool.tile([P, N], f32, tag="d2s", name=f"d2s_{k}")
                add_eng.tensor_tensor(d2s[:, :], sq[0][:, :], sq[1][:, :], OP.add)
                add_eng.tensor_tensor(d2s[:, :], d2s[:, :], sq[2][:, :], OP.add)
                u = wpool.tile([P, N], f32, tag="u", name=f"u_{k}")
                nc.scalar.activation(u[:, :], d2s[:, :], AT.Ln,
                                     bias=eps_b, scale=CUT)
                w_ = wpool.tile([P, N], f32, tag="w_", name=f"w_{k}")
                add_eng.tensor_tensor(w_[:, :], d2s[:, :], u[:, :], OP.add)
                g = wpool.tile([P, N], f32, tag="g", name=f"g_{k}")
                nc.scalar.activation(g[:, :], w_[:, :], AT.Exp,
                                     bias=nege49_b, scale=-1.0)

                # tn_a = (x_a[j] - x_a[i]) * g
                tn = [wpool.tile([P, N], f32, tag=f"tn{a}", name=f"tn{a}_{k}")
                      for a in range(3)]
                for a in range(3):
                    nc.vector.scalar_tensor_tensor(
                        tn[a][:, :], cb(a), col(_OFF_XYZI, k, a), g[:, :],
                        OP.subtract, OP.mult)

                # O_a[:, 3j+b] = (x_b[i] - x_b[j]) * tn_a = -g sep_a sep_b
                # accum S_ab = sum_j O_ab = -D_ab
                S = wpool.tile([P, 9], f32, tag="S", name=f"S_{k}")
                O = [bpool.tile([P, 3 * N], f32, tag=f"O{a}", name=f"O{a}_{k}")
                     for a in range(3)]
                for a in range(3):
                    for b in range(3):
                        nc.vector.scalar_tensor_tensor(
                            O[a][:, b:3 * N:3], ncb(b), col(_OFF_NXYZI, k, b),
                            tn[a][:, :], OP.subtract, OP.mult,
                            accum_out=S[:, 3 * a + b:3 * a + b + 1])
                # blend diagonal block: window j_loc = 128k+p
                for a in range(3):
                    for b in range(3):
                        W = O[a][:, 384 * k + b:384 * k + 384:3]
                        nc.vector.scalar_tensor_tensor(
                            W, inegt, S[:, 3 * a + b:3 * a + b + 1], W,
                            OP.mult, OP.add)
                for a in range(3):
                    nc.sync.dma_start(
                        out=out[384 * k + a:384 * k + 384:3, :], in_=O[a][:, :])
    nc.compile()
    return nc


def _get_nc():
    if "nc" not in _CACHE:
        _CACHE["nc"] = _build_nc()
    return _CACHE["nc"]


def _prep_inputs(coords, num_atoms):
    coords = np.asarray(coords, dtype=np.float32)
    num_atoms = np.asarray(num_atoms).astype(np.int64)
    xyzm = coords.reshape(B, N, 3).copy()
    for s in range(B):
        xyzm[s, int(num_atoms[s]):] = MASK
    in_maps = []
    for c in range(N_CORES):
        s, h = c // 2, c % 2
        i0 = ROWS * h
        perm = np.concatenate([
            np.arange(i0, i0 + ROWS), np.arange(0, i0), np.arange(i0 + ROWS, N)])
        cbm = xyzm[s][perm]                      # [N,3], own atoms first
        cb_arr = cbm.T.astype(np.float32)        # [3,N]
        xi = xyzm[s][i0:i0 + ROWS].astype(np.float32)       # [512,3]
        # xyzi-style cols: (k,a) -> value for partition p is x[i0+128k+p, a]
        xi_cols = xi.reshape(NB, P, 3).transpose(1, 0, 2).reshape(P, 3 * NB)
        cblk = np.empty((P, CF), dtype=np.float32)
        cblk[:, _OFF_CB:_OFF_CB + 3 * N] = np.broadcast_to(
            cb_arr.reshape(1, 3 * N), (P, 3 * N))
        cblk[:, _OFF_NCB:_OFF_NCB + 3 * N] = -cblk[:, _OFF_CB:_OFF_CB + 3 * N]
        cblk[:, _OFF_INEG:_OFF_INEG + P] = -np.eye(P, dtype=np.float32)
        cblk[:, _OFF_XYZI:_OFF_XYZI + 3 * NB] = xi_cols
        cblk[:, _OFF_NXYZI:_OFF_NXYZI + 3 * NB] = -xi_cols
        cblk[:, _OFF_M7:_OFF_M7 + 3 * NB] = -(xi_cols * np.float32(1.0 / 7.0))
        cblk[:, _OFF_EPS] = EPS
        cblk[:, _OFF_NEGE49] = -EPS / CUT
        in_maps.append({"consts": cblk})
    return in_maps


def _assemble_hess(results):
    hess = np.empty((B, 3 * N, 3 * N), dtype=np.float32)
    for c in range(N_CORES):
        s, h = c // 2, c % 2
        slab = results[c]["out"]
        rows = slice(1536 * h, 1536 * h + 1536)
        if h == 0:
            hess[s, rows] = slab
        else:
            hess[s, rows, 1536:] = slab[:, :1536]
            hess[s, rows, :1536] = slab[:, 1536:]
    return hess


def _top_mode_disp(hess):
    import jax
    import jax.numpy as jnp

    cpu = jax.devices("cpu")[0]
    n = hess.shape[1]

    def _top(H):
        w, V = jnp.linalg.eigh(H)
        idx = jnp.where(jnp.abs(w[-1]) > jnp.abs(w[0]), n - 1, 0)
        return w[idx], V[:, idx]

    with jax.default_device(cpu):
        lam, vec = jax.vmap(_top)(jnp.asarray(hess))
        disp = 3.0 * lam[:, None] * vec
    return np.asarray(disp, dtype=np.float32)


def kernel_with_results(coords, num_atoms, trace=False):
    from concourse import bass_utils

    nc = _get_nc()
    in_maps = _prep_inputs(coords, num_atoms)
    res = bass_utils.run_bass_kernel_spmd(
        nc, in_maps, core_ids=list(range(N_CORES)), trace=trace)
    hess = _assemble_hess(res.results)
    disp = _top_mode_disp(hess)
    return (hess, disp), res


def kernel(coords, num_atoms):
    (hess, disp), _ = kernel_with_results(coords, num_atoms, trace=False)
    return hess, disp
